# revision 1
# baseline (speedup 1.0000x reference)
"""IsoMaxPlus distance head on 8 NeuronCores.

out[n, c] = -|ds| * sqrt(max(2 - 2 * <f_n/|f_n|, p_c/|p_c|>, eps))

Data-parallel over the batch axis: features rows sharded 8 ways, prototypes and
distance_scale replicated (matches the sharding hint). The core matmul runs in
bf16 on the tensor engine (full rate; fp32 matmul is 4x slower), with fp32
norms/epilogue so the only precision loss is the bf16 rounding of the
normalized operands (~5e-5 relative on the output).
"""

import functools

import numpy as np

import jax
import jax.numpy as jnp
from jax.sharding import Mesh, NamedSharding, PartitionSpec as P

N_CORES = 8
EPS_NORM = 1e-12
EPS_SQ = 1e-12


def _normalize(x):
    n = jnp.sqrt(jnp.sum(x * x, axis=-1, keepdims=True))
    return x / jnp.maximum(n, EPS_NORM)


def _shard_fn(f, p, ds):
    # f: [N/8, D] local shard; p: [C, D] replicated; ds: [1] replicated
    fn = _normalize(f).astype(jnp.bfloat16)
    pn = _normalize(p).astype(jnp.bfloat16)
    sim = jax.lax.dot_general(
        fn, pn,
        dimension_numbers=(((1,), (1,)), ((), ())),
        preferred_element_type=jnp.float32,
    )
    sq = jnp.maximum(2.0 - 2.0 * sim, EPS_SQ)
    return -jnp.abs(ds[0]) * jnp.sqrt(sq)


@functools.cache
def _jitted():
    devices = jax.devices()[:N_CORES]
    mesh = Mesh(np.asarray(devices), ("core",))
    fn = jax.jit(
        jax.shard_map(
            _shard_fn,
            mesh=mesh,
            in_specs=(P("core"), P(), P()),
            out_specs=P("core"),
        ),
        in_shardings=(
            NamedSharding(mesh, P("core")),
            NamedSharding(mesh, P()),
            NamedSharding(mesh, P()),
        ),
    )
    return fn


def kernel(features, prototypes, distance_scale):
    features = np.ascontiguousarray(features, dtype=np.float32)
    prototypes = np.ascontiguousarray(prototypes, dtype=np.float32)
    distance_scale = np.ascontiguousarray(distance_scale, dtype=np.float32)
    out = _jitted()(features, prototypes, distance_scale)
    return np.asarray(jax.device_get(out)).astype(np.float32)



# revision 7
# speedup vs baseline: 44.2134x; 44.2134x over previous
"""IsoMaxPlus distance head on 8 NeuronCores — hand-written Bass/Tile kernel.

out[n, c] = -|ds| * sqrt(max(2 - 2 * <f_n/|f_n|, p_c/|p_c|>, eps))

Sharding (per the data-parallel hint): features rows are sharded 8 ways,
prototypes and distance_scale replicated; no collectives needed.

Per-core algorithm (N_loc=2048, D=2048, C=8192):
  1. Stream feature rows in natural layout [n, d]: compute row norms with a
     Square+accumulate activation pass (ScalarE), cast raw f to bf16 (VectorE)
     and stage to an internal DRAM buffer. Row norms are folded into the
     epilogue as a per-partition activation scale (-2/||f_n||), so features
     are never normalized explicitly.
  2. DMA-transpose (xbar) the staged bf16 features back as fT [d, n] — the
     contraction layout the PE needs. fT stays resident in SBUF (8 MB bf16).
  3. Stream prototype rows the same way, but normalized (per-row 1/||p_c||
     multiply on VectorE) before staging; DMA-transpose each 512-column block
     to pT [d, 512] right before its matmuls.
  4. Matmul: for each (c-block, m-tile): accumulate 16 K-tiles of
     [128,128]x[128,512] bf16 into one PSUM bank (fp32).
  5. Epilogue: one ScalarE activation  u = sqrt(psum * (-2/||f_n||) + 2)
     (reads PSUM directly), then one VectorE tensor_scalar  o = u * (-|ds|),
     then DMA the [128,512] fp32 tile to the output.

The bf16 matmul runs at full PE rate; fp32 epilogue. Only rounding of the
bf16 operands (~1e-4 relative on the output) differs from the reference.
"""

import functools
import sys

sys.path.insert(0, "/opt/trn_rl_repo")

import numpy as np

import jax
from jax.experimental.shard_map import shard_map
from jax.sharding import Mesh, NamedSharding, PartitionSpec as P

import concourse.bass as bass
import concourse.mybir as mybir
import concourse.tile as tile
from concourse.bass2jax import bass_jit

N_CORES = 8
PART = 128
F32 = mybir.dt.float32
BF16 = mybir.dt.bfloat16
AF = mybir.ActivationFunctionType
ALU = mybir.AluOpType


def build_iso_kernel(tc, out, f, p, ds):
    """Emit the per-core kernel. out: [N_loc, C] f32; f: [N_loc, D] f32;
    p: [C, D] f32; ds: [1] f32. All APs over DRAM tensors."""
    nc = tc.nc
    n_loc, d = f.shape
    c, d2 = p.shape
    assert d == d2 and n_loc % PART == 0 and d % PART == 0
    kt = d // PART            # K tiles along contraction
    mt = n_loc // PART        # M tiles (feature rows)
    cb = min(512, c)          # c-block width (one PSUM bank)
    ncb = c // cb
    cj = cb // PART           # 128-row chunks per c-block

    import contextlib

    with contextlib.ExitStack() as ctx:
        consts = ctx.enter_context(tc.tile_pool(name="consts", bufs=1))
        nat = ctx.enter_context(tc.tile_pool(name="nat", bufs=3))
        sqs = ctx.enter_context(tc.tile_pool(name="sqs", bufs=2))
        cast = ctx.enter_context(tc.tile_pool(name="cast", bufs=3))
        small = ctx.enter_context(tc.tile_pool(name="small", bufs=8))
        ftp = ctx.enter_context(tc.tile_pool(name="ftp", bufs=1))
        ptp = ctx.enter_context(tc.tile_pool(name="ptp", bufs=2))
        upool = ctx.enter_context(tc.tile_pool(name="upool", bufs=4))
        opool = ctx.enter_context(tc.tile_pool(name="opool", bufs=4))
        pspool = ctx.enter_context(tc.tile_pool(name="pspool", bufs=4, space="PSUM"))
        dram_f = ctx.enter_context(tc.tile_pool(name="dram_f", bufs=1, space="DRAM"))
        dram_p = ctx.enter_context(tc.tile_pool(name="dram_p", bufs=3, space="DRAM"))

        # ---- constants: negds[p] = -|ds| broadcast over partitions ----
        ds_b = consts.tile([PART, 1], F32, tag="ds_b")
        ds_bcast = bass.AP(tensor=ds.tensor, offset=ds.offset, ap=[[0, PART], [1, 1]])
        nc.gpsimd.dma_start(out=ds_b, in_=ds_bcast)
        # scale_all[:, m] = -2 / ||f_row||
        scale_all = consts.tile([PART, mt], F32, tag="scale_all")
        bias2 = consts.tile([PART, 1], F32, tag="bias2")
        nc.vector.memset(bias2, 2.0)
        zero = consts.tile([PART, 1], F32, tag="zero")
        nc.vector.memset(zero, 0.0)
        absds = consts.tile([PART, 1], F32, tag="absds")
        nc.scalar.activation(out=absds, in_=ds_b, func=AF.Abs, bias=zero)
        negds = consts.tile([PART, 1], F32, tag="negds")
        nc.vector.tensor_scalar_mul(negds, absds, -1.0)

        # ---- feature pass: norms + bf16 staging ----
        fbf = dram_f.tile([n_loc, d], BF16, tag="fbf")
        for i in range(mt):
            fnat = nat.tile([PART, d], F32, tag="nat")
            nc.sync.dma_start(out=fnat, in_=f[i * PART:(i + 1) * PART, :])
            ss = small.tile([PART, 1], F32, tag="small")
            sq = sqs.tile([PART, d], F32, tag="sq")
            nc.scalar.activation(out=sq, in_=fnat, func=AF.Square, accum_out=ss)
            fc = cast.tile([PART, d], BF16, tag="cast")
            nc.vector.tensor_copy(out=fc, in_=fnat)
            nc.sync.dma_start(out=fbf[i * PART:(i + 1) * PART, :], in_=fc)
            nrm = small.tile([PART, 1], F32, tag="small")
            nc.scalar.activation(out=nrm, in_=ss, func=AF.Sqrt, bias=zero)
            inv = small.tile([PART, 1], F32, tag="small")
            nc.vector.reciprocal(inv, nrm)
            nc.vector.tensor_scalar(
                out=scale_all[:, i:i + 1], in0=inv, scalar1=-2.0, scalar2=None,
                op0=ALU.mult,
            )

        # ---- transposed resident features fT[d, n] ----
        fT = ftp.tile([PART, kt, n_loc], BF16, tag="fT")
        for k in range(kt):
            nc.sync.dma_start_transpose(fT[:, k, :], fbf[:, k * PART:(k + 1) * PART])

        # ---- main loop over c-blocks ----
        for b in range(ncb):
            pbf = dram_p.tile([cb, d], BF16, tag="pbf")
            for j in range(cj):
                r0 = b * cb + j * PART
                pnat = nat.tile([PART, d], F32, tag="nat")
                nc.sync.dma_start(out=pnat, in_=p[r0:r0 + PART, :])
                ssp = small.tile([PART, 1], F32, tag="small")
                sqp = sqs.tile([PART, d], F32, tag="sq")
                nc.scalar.activation(out=sqp, in_=pnat, func=AF.Square, accum_out=ssp)
                nrmp = small.tile([PART, 1], F32, tag="small")
                nc.scalar.activation(out=nrmp, in_=ssp, func=AF.Sqrt, bias=zero)
                invp = small.tile([PART, 1], F32, tag="small")
                nc.vector.reciprocal(invp, nrmp)
                pc = cast.tile([PART, d], BF16, tag="cast")
                nc.vector.tensor_scalar_mul(pc, pnat, invp)
                nc.sync.dma_start(out=pbf[j * PART:(j + 1) * PART, :], in_=pc)

            pT = ptp.tile([PART, kt, cb], BF16, tag="pT")
            for k in range(kt):
                nc.sync.dma_start_transpose(pT[:, k, :], pbf[:, k * PART:(k + 1) * PART])

            for m in range(mt):
                ps = pspool.tile([PART, cb], F32, tag="ps")
                for k in range(kt):
                    nc.tensor.matmul(
                        ps,
                        lhsT=fT[:, k, m * PART:(m + 1) * PART],
                        rhs=pT[:, k, :],
                        start=(k == 0),
                        stop=(k == kt - 1),
                    )
                u = upool.tile([PART, cb], F32, tag="u")
                nc.scalar.activation(
                    out=u, in_=ps, func=AF.Sqrt,
                    bias=bias2, scale=scale_all[:, m:m + 1],
                )
                oo = opool.tile([PART, cb], F32, tag="o")
                nc.vector.tensor_scalar_mul(oo, u, negds)
                nc.sync.dma_start(
                    out=out[m * PART:(m + 1) * PART, b * cb:(b + 1) * cb], in_=oo,
                )


@bass_jit
def _iso_bass(nc, f, p, ds):
    out = nc.dram_tensor(
        "out", [f.shape[0], p.shape[0]], F32, kind="ExternalOutput"
    )
    with tile.TileContext(nc) as tc:
        build_iso_kernel(tc, out[:], f[:], p[:], ds[:])
    return out


@functools.cache
def _jitted():
    devices = jax.devices()[:N_CORES]
    mesh = Mesh(np.asarray(devices), ("core",))
    fn = jax.jit(
        shard_map(
            _iso_bass,
            mesh=mesh,
            in_specs=(P("core"), P(), P()),
            out_specs=P("core"),
            check_rep=False,
        )
    )
    return fn, mesh


def kernel(features, prototypes, distance_scale):
    features = np.ascontiguousarray(features, dtype=np.float32)
    prototypes = np.ascontiguousarray(prototypes, dtype=np.float32)
    distance_scale = np.ascontiguousarray(distance_scale, dtype=np.float32)
    fn, mesh = _jitted()
    f = jax.device_put(features, NamedSharding(mesh, P("core")))
    p = jax.device_put(prototypes, NamedSharding(mesh, P()))
    ds = jax.device_put(distance_scale, NamedSharding(mesh, P()))
    out = fn(f, p, ds)
    return np.asarray(jax.device_get(out)).astype(np.float32)
